# revision 63
# baseline (speedup 1.0000x reference)
# Causal self-attention kernel for 8 Trainium2 NeuronCores.
#
# Problem (hardcoded): B=2, S=2048, D=1024, H=16 heads of dk=64.
#   q,k,v = x @ W.T + b (torch Linear), per-head causal softmax attention,
#   out[b,s,:] = concat_h(attn_h @ v_h). No output projection.
#
# Sharding: 8 cores = 2 batches x 4 head-groups. Core c handles batch c//4
# and heads [4*(c%4), 4*(c%4)+4) => output channels [256*(c%4), +256).
# No cross-device communication.
#
# v2 design (engine-balance driven; sim-profiled):
#   - Host passes xT = x[b].T in bf16: no on-device x transposes at all.
#     Weights/v/attn-weights in bf16 (same PE rate as f32r, ~0.2% err).
#   - QK^T scores in fp8(e4m3) DoubleRow: 0.5 cyc/row, 2x the PE rate.
#     q/k are written to fp8 stage tiles by the projection bias-add, then
#     an SBUF->SBUF DMA repacks each head's [64, S] into the DoubleRow
#     pair layout [32, 2, S] (contraction dk = 32*i + partition).
#   - v carries a ones column per head so PV also produces softmax
#     denominators (row 64 of the accumulator).
#   - Attention per (head, sq-half): key-block j outer; scoresT[sk,sq] on
#     PE, multiplicative 0/1 causal mask on the diagonal block (gpsimd),
#     one exp per (j) segment on ACT, PV accumulates outT_aug[65, 1024]
#     in PSUM across j (PV deferred one j so PE runs ahead of ACT).
#   - Tail per (head, half): copy accumulator to SBUF (bf16), PE-transpose
#     back incl. sums row, reciprocal (DVE) + per-partition scale (gpsimd,
#     which is otherwise idle) into the output staging tile.
#   - Emission interleaving: only head 0's first-half prerequisites are
#     emitted up front; remaining projection/repack units are drip-fed
#     into the attention phase, with targeted drains at each head-half
#     boundary (filler units fill PE while ACT grinds through exp).

import numpy as np
import ml_dtypes

B, S, D, H = 2, 2048, 1024, 16
DK = D // H            # 64
NCORES = 8
HPC = 4                # heads per core
E = HPC * DK           # 256 output channels per core
EA = HPC * (DK + 1)    # 260 augmented v width (ones col per head)
P = 128
NSB = S // P           # 16 s-blocks
NDC = D // P           # 8 d-chunks
HALF = 1024

USE_FP8_QK = True

_cache = {}


def _build_module():
    import concourse.bacc as bacc
    import concourse.mybir as mybir
    import concourse.tile as tile

    f32 = mybir.dt.float32
    f32r = mybir.dt.float32r
    bf16 = mybir.dt.bfloat16
    f8 = mybir.dt.float8e4
    Exp = mybir.ActivationFunctionType.Exp
    DR = mybir.MatmulPerfMode.DoubleRow

    nc = bacc.Bacc("TRN2", target_bir_lowering=False, debug=False)

    # xt is host-packed as [p, sc, dc, 512] so each DMA slice is one long
    # contiguous run per partition (cheap SP descriptor generation).
    xt_d = nc.dram_tensor("xt", [P, 4, NDC, 512], bf16, kind="ExternalInput")
    wq_d = nc.dram_tensor("wq_t", [D, E], bf16, kind="ExternalInput")
    wk_d = nc.dram_tensor("wk_t", [D, E], bf16, kind="ExternalInput")
    wv_d = nc.dram_tensor("wv_t", [D, EA], bf16, kind="ExternalInput")
    # consts blob: [mask(128) | ident(128) | bqc,bkc as f32 bits(8)] per part
    cst_d = nc.dram_tensor("cst", [P, 264], bf16, kind="ExternalInput")
    bv_d = nc.dram_tensor("bv", [1, EA], bf16, kind="ExternalInput")
    out_d = nc.dram_tensor("out", [S, E], bf16, kind="ExternalOutput")

    qk_dt = f8 if USE_FP8_QK else bf16

    with tile.TileContext(nc) as tc:
        with (
            tc.tile_pool(name="consts", bufs=1) as consts,
            tc.tile_pool(name="xt", bufs=1) as xtp,
            tc.tile_pool(name="qkv", bufs=1) as qkv,
            tc.tile_pool(name="outst", bufs=1) as outst,
            tc.tile_pool(name="pp", bufs=2, space="PSUM") as pp,
            tc.tile_pool(name="psc", bufs=2, space="PSUM") as pscp,
            tc.tile_pool(name="pacc", bufs=1, space="PSUM") as paccp,
            tc.tile_pool(name="attn", bufs=4) as attnp,
            tc.tile_pool(name="otn", bufs=2) as otnp,
        ):
            # ---- tiles ----
            wq_sb = consts.tile([P, NDC, E], bf16, tag="wq")
            wk_sb = consts.tile([P, NDC, E], bf16, tag="wk")
            wv_sb = consts.tile([P, NDC, EA], bf16, tag="wv")
            cst_sb = consts.tile([P, 264], bf16, tag="cst")
            bv_sb = consts.tile([1, EA], bf16, tag="bv")
            mask_sb = cst_sb[:, 0:P]
            ident_sb = cst_sb[:, P:2 * P]
            ones_sb = mask_sb[0:1, :]      # mask row 0 is all-ones
            qkbias = cst_sb[:, 256:264].bitcast(f32)   # [P, 4] f32
            bqc_sb = qkbias[:, 0:2]
            bkc_sb = qkbias[:, 2:4]

            xT = xtp.tile([P, 4, NDC, 512], bf16, tag="xT")

            # q/k stage: projection output layout [channel(128), eb, s]
            qstage = qkv.tile([P, 2, S], qk_dt, tag="qstage")
            kstage = qkv.tile([P, 2, S], qk_dt, tag="kstage")
            if USE_FP8_QK:
                # DoubleRow pair layout: two heads per 64-partition tile
                # (matmul base partition must be 0/32/64): head h lives in
                # tile h//2 at partitions 32*(h%2)..+32; axis1 i, axis2 s;
                # contraction index dk = 32*i + r for head h.
                qdr = [qkv.tile([64, 2, S], f8, tag=f"qdr{t}", name=f"qdr{t}")
                       for t in range(2)]
                kdr = [qkv.tile([64, 2, S], f8, tag=f"kdr{t}", name=f"kdr{t}")
                       for t in range(2)]
            v_sb = qkv.tile([P, NSB, EA], bf16, tag="v")
            out_sb = outst.tile([P, NSB, E], bf16, tag="out")

            # ---- DMAs (first-needed first; the DMA device is serial, so
            # late-needed transfers are emitted after the prefix's compute) --
            def emit_xt_dma(sc, ha):
                nc.sync.dma_start(
                    out=xT[:, sc, :, ha * 256:(ha + 1) * 256],
                    in_=xt_d[:, sc, :, ha * 256:(ha + 1) * 256],
                )

            def emit_late_dmas():
                nc.sync.dma_start(
                    out=wv_sb, in_=wv_d[:].rearrange("(c p) e -> p c e", p=P))
                nc.sync.dma_start(out=bv_sb, in_=bv_d[:])
                emit_w_dma(wq_sb, wq_d, 1)
                emit_w_dma(wk_sb, wk_d, 1)
                nc.sync.dma_start(out=xT[:, 2], in_=xt_d[:, 2])
                nc.sync.dma_start(out=xT[:, 3], in_=xt_d[:, 3])

            def emit_w_dma(w_sb, w_d, eb):
                nc.sync.dma_start(
                    out=w_sb[:, :, eb * P:(eb + 1) * P],
                    in_=w_d[:, eb * P:(eb + 1) * P]
                    .rearrange("(c p) e -> p c e", p=P),
                )

            emit_w_dma(wq_sb, wq_d, 0)
            nc.sync.dma_start(out=cst_sb, in_=cst_d[:])
            emit_xt_dma(0, 0)
            emit_w_dma(wk_sb, wk_d, 0)
            emit_xt_dma(0, 1)
            emit_xt_dma(1, 0)
            emit_xt_dma(1, 1)

            # ---- phase-A units ----
            # qk projection quarter-units: 256 output columns each, fully
            # self-contained (own psum tile + bias-add) so the filler can
            # pace PE in ~850ns grains without cross-unit PSUM hazards.
            def emit_qk_proj_q(which, eb, sq):
                w_sb = wq_sb if which == 0 else wk_sb
                bc = bqc_sb if which == 0 else bkc_sb
                dst = qstage if which == 0 else kstage
                sc, so = sq // 2, (sq % 2) * 256
                ps = pp.tile([P, 512], f32, tag="pp", name="ps")
                ps = ps[:, 0:256]
                for dc in range(NDC):
                    nc.tensor.matmul(
                        ps,
                        lhsT=w_sb[:, dc, eb * P:(eb + 1) * P],
                        rhs=xT[:, sc, dc, so:so + 256],
                        start=(dc == 0),
                        stop=(dc == NDC - 1),
                    )
                nc.vector.tensor_scalar_add(
                    dst[:, eb, sq * 256:(sq + 1) * 256], ps, bc[:, eb:eb + 1]
                )

            def emit_v_proj(sb):
                ps = pp.tile([P, 512], f32, tag="pp", name="ps")
                pv = ps[:, :EA]
                so = (sb % 4) * P
                for dc in range(NDC):
                    nc.tensor.matmul(
                        pv,
                        lhsT=xT[:, sb // 4, dc, so:so + P],
                        rhs=wv_sb[:, dc, :],
                        start=(dc == 0),
                        stop=False,
                    )
                nc.tensor.matmul(
                    pv,
                    lhsT=ones_sb[0:1, :],
                    rhs=bv_sb[0:1, :],
                    start=False,
                    stop=True,
                )
                nc.vector.tensor_copy(v_sb[:, sb, :], pv)

            def emit_repack(which, h, sh):
                # stage [64ch, S] of head h -> DR pair layout [32, 2, S].
                # One SWDGE DMA (Pool engine: skips the contended HWDGE).
                # DMA pairs elements in AP iteration order: dst (r, i, s)
                # nests i inside r, so dst[r, i] <- src channel 2r+i. The DR
                # contraction split is therefore dk = 2r+i — fine, since q
                # and k use the same mapping.
                src = qstage if which == 0 else kstage
                dst = (qdr if which == 0 else kdr)[h // 2]
                eb, hl = h // 2, h % 2
                po = 32 * (h % 2)
                lo = sh * HALF
                nc.gpsimd.dma_start(
                    out=dst[po:po + 32, :, lo:lo + HALF],
                    in_=src[64 * hl:64 * hl + 64, eb, lo:lo + HALF],
                )

            # ---- attention ----
            V_MARK = {}      # v s-block -> absolute filler unit index
            RP_MARK = {}     # (h, sh) -> absolute filler unit index
            drain_to_mark = [None]  # bound after the filler is built

            def attn_head_half(h, half, jhook=None, per_block_dma=False,
                               no_dr=False):
                lo = half * HALF
                hi = lo + HALF
                if (h, half) in RP_MARK:
                    drain_to_mark[0](RP_MARK[(h, half)])
                pacc = paccp.tile([65, HALF], f32, tag="pacc")

                def emit_pv(j, at):
                    if j in V_MARK:
                        drain_to_mark[0](V_MARK[j])
                    sb0 = max(j * P, lo)
                    lhsT_v = v_sb[:, j, h * 65:(h + 1) * 65]
                    m = sb0
                    while m < hi:
                        w = min(512 - (m - lo) % 512, hi - m)
                        bank = (m - lo) // 512
                        j_last = min((lo + 512 * (bank + 1)) // P - 1, hi // P - 1)
                        nc.tensor.matmul(
                            pacc[:, m - lo:m - lo + w],
                            lhsT=lhsT_v,
                            rhs=at[:, m - sb0:m - sb0 + w],
                            start=(j == 0),
                            stop=(j == j_last),
                        )
                        m += w

                # PV is deferred TWO iterations so it never stalls PE on the
                # exp->mask latency of its own j (iter period stays ACT-paced)
                pending = []
                for j in range(hi // P):
                    ko = j * P
                    sb0 = max(ko, lo)
                    segw = hi - sb0
                    ps = pscp.tile([P, HALF], f32, tag="sc")
                    m = 0
                    while m < segw:
                        w = min(512, segw - m)
                        if USE_FP8_QK and not no_dr:
                            po = 32 * (h % 2)
                            nc.tensor.matmul(
                                ps[:, m:m + w],
                                lhsT=kdr[h // 2][po:po + 32, :, ko:ko + P],
                                rhs=qdr[h // 2][po:po + 32, :, sb0 + m:sb0 + m + w],
                                perf_mode=DR,
                                start=True,
                                stop=True,
                            )
                        else:
                            # same fp8 data, 1 cyc/row: used for (0,0) so its
                            # scores don't wait on the repack DMA at startup
                            hl, eb = h % 2, h // 2
                            nc.tensor.matmul(
                                ps[:, m:m + w],
                                lhsT=kstage[64 * hl:64 * hl + DK, eb, ko:ko + P],
                                rhs=qstage[64 * hl:64 * hl + DK, eb,
                                           sb0 + m:sb0 + m + w],
                                start=True,
                                stop=True,
                            )
                        m += w
                    at = attnp.tile([P, HALF], bf16, tag="at")
                    nc.scalar.activation(
                        out=at[:, :segw], in_=ps[:, :segw], func=Exp, scale=0.125
                    )
                    if ko >= lo:
                        nc.vector.tensor_mul(at[:, 0:P], at[:, 0:P], mask_sb)
                    if len(pending) >= 2:
                        emit_pv(*pending.pop(0))
                    if jhook is not None:
                        jhook(j)
                    pending.append((j, at))
                for p in pending:
                    emit_pv(*p)

                # tail: normalize + transpose back + stage. The PSUM->SBUF
                # copy runs per 128-block so the transpose chain starts
                # ~260ns in instead of ~1.2us (PE would idle behind it).
                otn = otnp.tile([65, HALF], bf16, tag="otn")
                for il in range(HALF // P):
                    i = half * 8 + il
                    nc.vector.tensor_copy(
                        otn[:, il * P:(il + 1) * P],
                        pacc[:, il * P:(il + 1) * P],
                    )
                    pot = pp.tile([P, 1024], bf16, tag="pp", name="pot")
                    nc.tensor.transpose(
                        pot[:, 0:65], otn[:, il * P:(il + 1) * P],
                        ident_sb[0:65, 0:65]
                    )
                    linv = otnp.tile([P, 1], f32, tag="linv")
                    nc.vector.reciprocal(linv, pot[:, DK:DK + 1])
                    nc.vector.tensor_scalar_mul(
                        out_sb[:, i, h * DK:(h + 1) * DK], pot[:, 0:DK], linv
                    )
                    if jhook is not None:
                        jhook(None)
                    if per_block_dma and il in (3, 7):
                        i0 = half * 8 + il - 3
                        nc.sync.dma_start(
                            out=out_d[i0 * P:(i0 + 4) * P, :]
                            .rearrange("(i p) e -> p i e", p=P),
                            in_=out_sb[:, i0:i0 + 4, :],
                        )

            # ---- schedule ----
            # prefix: head 0 first-half prerequisites (q,k eb0 cols 0:1024).
            # (0,0) reads the stage tiles directly (no_dr) so no repack here.
            emit_qk_proj_q(0, 0, 0)
            emit_qk_proj_q(0, 0, 1)
            emit_qk_proj_q(1, 0, 0)
            emit_qk_proj_q(1, 0, 1)
            emit_qk_proj_q(0, 0, 2)
            emit_qk_proj_q(0, 0, 3)
            emit_qk_proj_q(1, 0, 2)
            emit_qk_proj_q(1, 0, 3)
            emit_late_dmas()

            # filler units (est PE-ns, fn), ordered by first need
            QK_NS, V_NS, RP_NS = 860, 1090, 50
            filler = []

            def unit(est, fn, *a):
                filler.append((est, lambda: fn(*a)))

            for sb in range(0, 8):
                unit(V_NS, emit_v_proj, sb)
                V_MARK[sb] = len(filler)
            if USE_FP8_QK:
                unit(RP_NS, emit_repack, 0, 1, 0)
                unit(RP_NS, emit_repack, 1, 1, 0)
            RP_MARK[(1, 0)] = len(filler)
            for sq in range(4, 8):
                unit(QK_NS, emit_qk_proj_q, 0, 0, sq)
                unit(QK_NS, emit_qk_proj_q, 1, 0, sq)
            if USE_FP8_QK:
                unit(RP_NS, emit_repack, 1, 0, 0)  # k h0 sh0 ((0,0) ran no_dr)
                unit(RP_NS, emit_repack, 0, 0, 1)
                unit(RP_NS, emit_repack, 1, 0, 1)
            RP_MARK[(0, 1)] = len(filler)
            if USE_FP8_QK:
                unit(RP_NS, emit_repack, 0, 1, 1)
                unit(RP_NS, emit_repack, 1, 1, 1)
            RP_MARK[(1, 1)] = len(filler)
            for sb in range(8, NSB):
                unit(V_NS, emit_v_proj, sb)
                V_MARK[sb] = len(filler)
            for sq in range(4):
                unit(QK_NS, emit_qk_proj_q, 0, 1, sq)
                unit(QK_NS, emit_qk_proj_q, 1, 1, sq)
            if USE_FP8_QK:
                unit(RP_NS, emit_repack, 0, 2, 0)
                unit(RP_NS, emit_repack, 1, 2, 0)
            RP_MARK[(2, 0)] = len(filler)
            if USE_FP8_QK:
                unit(RP_NS, emit_repack, 0, 3, 0)
                unit(RP_NS, emit_repack, 1, 3, 0)
            RP_MARK[(3, 0)] = len(filler)
            for sq in range(4, 8):
                unit(QK_NS, emit_qk_proj_q, 0, 1, sq)
                unit(QK_NS, emit_qk_proj_q, 1, 1, sq)
            if USE_FP8_QK:
                unit(RP_NS, emit_repack, 0, 2, 1)
                unit(RP_NS, emit_repack, 1, 2, 1)
            RP_MARK[(2, 1)] = len(filler)
            if USE_FP8_QK:
                unit(RP_NS, emit_repack, 0, 3, 1)
                unit(RP_NS, emit_repack, 1, 3, 1)
            RP_MARK[(3, 1)] = len(filler)

            ndrained = [0]
            budget = [0.0]

            def drain_one():
                est, fn = filler.pop(0)
                fn()
                ndrained[0] += 1
                budget[0] -= est

            def mkhook(rate):
                def hook(j):
                    budget[0] += rate
                    while filler and budget[0] >= filler[0][0]:
                        drain_one()
                return hook

            def drain_to(mark):
                while ndrained[0] < mark and filler:
                    drain_one()

            drain_to_mark[0] = drain_to

            hook = mkhook(430)   # first halves
            hook2 = mkhook(450)  # second halves
            attn_head_half(0, 0, jhook=hook, no_dr=True)
            attn_head_half(1, 0, jhook=hook)
            attn_head_half(0, 1, jhook=hook2)
            attn_head_half(1, 1, jhook=hook2)
            attn_head_half(2, 0, jhook=hook)
            attn_head_half(3, 0, jhook=hook)
            nc.sync.dma_start(
                out=out_d[0:8 * P, :].rearrange("(i p) e -> p i e", p=P),
                in_=out_sb[:, 0:8, :],
            )
            attn_head_half(2, 1, jhook=hook2)
            attn_head_half(3, 1, jhook=hook2, per_block_dma=True)

    nc.compile()
    return nc


def _prep_core_inputs(inputs, c):
    bf16 = ml_dtypes.bfloat16
    x = np.asarray(inputs["x"], dtype=np.float32)
    b, hg = c // HPC, c % HPC
    e0 = hg * E

    wq = np.asarray(inputs["Wq"], dtype=np.float32)
    wk = np.asarray(inputs["Wk"], dtype=np.float32)
    wv = np.asarray(inputs["Wv"], dtype=np.float32)
    bq = np.asarray(inputs["bq"], dtype=np.float32)
    bk = np.asarray(inputs["bk"], dtype=np.float32)
    bv = np.asarray(inputs["bv"], dtype=np.float32)

    # xt host-packed [p, sc, dc, 512]: xt[p,sc,c,s'] = x[b][sc*512+s', c*128+p]
    xt = np.ascontiguousarray(
        x[b].T.astype(bf16).reshape(NDC, P, 4, 512).transpose(1, 2, 0, 3)
    )
    wq_t = np.ascontiguousarray(wq[e0:e0 + E, :].T).astype(bf16)  # [D, E]
    wk_t = np.ascontiguousarray(wk[e0:e0 + E, :].T).astype(bf16)
    wv_t = np.zeros((D, EA), dtype=np.float32)
    bv_a = np.zeros((1, EA), dtype=np.float32)
    for lh in range(HPC):
        cols = slice(lh * 65, lh * 65 + DK)
        rows = slice(e0 + lh * DK, e0 + lh * DK + DK)
        wv_t[:, cols] = wv[rows, :].T
        bv_a[0, cols] = bv[rows]
        bv_a[0, lh * 65 + DK] = 1.0                              # ones column

    cst = np.zeros((P, 264), dtype=bf16)
    cst[:, 0:P] = np.where(
        np.arange(P)[None, :] >= np.arange(P)[:, None], 1.0, 0.0
    ).astype(bf16)
    cst[:, P:2 * P] = np.eye(P, dtype=np.float32).astype(bf16)
    biases = np.ascontiguousarray(np.concatenate(
        [bq[e0:e0 + E].reshape(2, P).T, bk[e0:e0 + E].reshape(2, P).T], axis=1
    ).astype(np.float32))                      # [P, 4] f32
    cst[:, 256:264] = biases.view(np.uint16).view(bf16)

    return {
        "xt": xt,
        "wq_t": wq_t,
        "wk_t": wk_t,
        "wv_t": wv_t.astype(bf16),
        "bv": bv_a.astype(bf16),
        "cst": cst,
    }


def kernel(**inputs):
    from concourse.bass_utils import run_bass_kernel_spmd

    if "nc" not in _cache:
        _cache["nc"] = _build_module()
    nc = _cache["nc"]

    in_maps = [_prep_core_inputs(inputs, c) for c in range(NCORES)]
    res = run_bass_kernel_spmd(nc, in_maps, core_ids=list(range(NCORES)))

    out = np.empty((B, S, D), dtype=np.float32)
    for c in range(NCORES):
        b, hg = c // HPC, c % HPC
        out[b, :, hg * E:(hg + 1) * E] = res.results[c]["out"].astype(np.float32)
    return out
